# revision 1
# baseline (speedup 1.0000x reference)
"""GCN (2-layer GCNConv + ReLU) Trainium2 Bass kernel, 8-core SPMD.

Math: with A = D^-1/2 (Adj + I) D^-1/2 (self loops added, symmetric norm),
  h1 = relu(A @ (x @ W1) + b1)
  h2 = relu(A @ (h1 @ W2) + b2)
Factorization used on device (norm_e = dinv[src]*dinv[dst], segment_sum
commutes with the weight GEMM):
  table1 = dinv * x                     (host, bf16)
  PT[t]  = selfrows[t] (identity mm) + sum_e table[src_e] x onehot(slot_e)
  psum   = PT^T @ W + sqrtdeg*b                     (PE)
  layer1 out (= table2 = dinv*h1) = relu(dinv^2 * psum)   (ACT, bf16)
  layer2 out (= h2)               = relu(dinv   * psum)   (ACT, f32)
Between layers an AllGather shares table2 across the 8 cores.

Sharding: nodes split in contiguous ranges; node g lives on device g//SHARD,
local row g%SHARD; per-device rows padded to SHARD_PAD (mult of 128).
Non-self edges partitioned by dst device, sorted by (dst tile, src row) and
cut into 128-edge chunks; each chunk is gathered from one 32768-row window
of the table (overlapping windows, chosen per chunk from the data so the
int16 gather indices fit). Self loops are handled without any gather: an
identity-rhs matmul adds the local shard's 128 rows per dst tile.
"""

import os
import sys

sys.path.insert(0, "/opt/trn_rl_repo")

import numpy as np
import ml_dtypes

from concourse import bass, bacc, mybir, tile
from concourse.bass_utils import run_bass_kernel_spmd

NCORES = 8
P = 128
WROWS = 32768          # int16 gather index limit (rows per window)
TB = 8                 # tiles per gather batch
MAXCALL = 48           # max chunks per dma_gather call (SWDGE ring safety)

LAST_RESULTS = None    # test harness peeks at this for profile info


def _ceil_div(a, b):
    return -(-a // b)


def _prep(x, W1, b1, W2, b2, edge_index):
    """Host-side graph preprocessing -> (meta, per-device input maps)."""
    N, D = x.shape
    SHARD = _ceil_div(N, NCORES)
    SHARD_PAD = _ceil_div(SHARD, P) * P
    NT = SHARD_PAD // P
    V = NCORES * SHARD_PAD

    # gather windows (overlapping, data-assigned per chunk)
    if V <= WROWS:
        bases = np.array([0], dtype=np.int64)
        wrows = V
    else:
        nw = max(2, _ceil_div(V - WROWS, 16896) + 1)
        stride = _ceil_div(V - WROWS, nw - 1)
        bases = np.array(
            sorted({min(w * stride, V - WROWS) for w in range(nw)}), dtype=np.int64
        )
        wrows = WROWS
    NWIN = len(bases)

    src = np.asarray(edge_index[0], dtype=np.int64)
    dst = np.asarray(edge_index[1], dtype=np.int64)

    deg = (np.bincount(dst, minlength=N) + 1).astype(np.float64)  # + self loop
    dinv = 1.0 / np.sqrt(deg)

    # in-degree-balanced node -> table-row permutation (snake deal over all
    # NCORES*NT 128-node bins) so every (device, tile) has ~equal edge count
    # and chunk counts hit the ceil with minimal padding.
    NBINS = NCORES * NT
    indeg = np.bincount(dst, minlength=N)
    by_deg = np.argsort(-indeg, kind="stable")
    node_row = np.empty(N, dtype=np.int64)
    bin_fill = np.zeros(NBINS, dtype=np.int64)
    fwd = np.arange(NBINS)
    for r in range(_ceil_div(N, NBINS)):
        sl_nodes = by_deg[r * NBINS : (r + 1) * NBINS]
        bins = fwd[: len(sl_nodes)] if r % 2 == 0 else (NBINS - 1 - fwd)[: len(sl_nodes)]
        d_of = bins % NCORES
        t_of = bins // NCORES
        node_row[sl_nodes] = d_of * SHARD_PAD + t_of * P + bin_fill[bins]
        bin_fill[bins] += 1
    assert bin_fill.max() <= P

    # layer-1 gather table: dinv * x at permuted rows, padded to V rows, bf16
    table1 = np.zeros((V, D), dtype=ml_dtypes.bfloat16)
    scaled = (x.astype(np.float64) * dinv[:, None]).astype(np.float32)
    table1[node_row] = scaled.astype(ml_dtypes.bfloat16)

    # edges -> (device, tile, slot, srow), sorted by (dev, tile, srow)
    drow = node_row[dst]
    dev = drow // SHARD_PAD
    ltile = (drow % SHARD_PAD) // P
    slot = (drow % P).astype(np.int64)
    srow = node_row[src]
    order = np.lexsort((srow, ltile, dev))
    s_dev, s_tile = dev[order], ltile[order]
    s_srow, s_slot = srow[order], slot[order]

    key = s_dev * NT + s_tile
    cnt = np.bincount(key, minlength=NCORES * NT).reshape(NCORES, NT)
    NCH = _ceil_div(cnt, P).max(axis=0)  # [NT] chunks per tile (can be 0)
    gstart = np.zeros(NCORES * NT + 1, dtype=np.int64)
    np.cumsum(cnt.reshape(-1), out=gstart[1:])

    # per-(tile, chunk) window assignment from LO/HI over devices
    def srows_of(d, t, j):
        a = gstart[d * NT + t]
        c = cnt[d, t]
        lo_i, hi_i = j * P, min((j + 1) * P, c)
        if lo_i >= hi_i:
            return None
        return s_srow[a + lo_i : a + hi_i]

    win_of = {}
    for t in range(NT):
        for j in range(int(NCH[t])):
            lo, hi = V, -1
            for d in range(NCORES):
                rs = srows_of(d, t, j)
                if rs is not None:
                    lo = min(lo, int(rs[0]))
                    hi = max(hi, int(rs[-1]))
            ok = np.where((bases <= lo) & (hi < bases + wrows))[0]
            assert len(ok) > 0, f"no window fits tile {t} chunk {j}: [{lo},{hi}]"
            mid = (lo + hi) / 2
            win_of[(t, j)] = int(ok[np.argmin(np.abs(bases[ok] + wrows / 2 - mid))])

    # batch / call / column layout
    NB = _ceil_div(NT, TB)
    batch_meta = []   # per batch: (t0, t1, calls, slot_off, chb)
    col_of = {}       # (t, j) -> column within its batch's g tile
    idx_parts, slot_parts = [], []
    idx_off = 0
    slot_off = 0
    chb_max = 0
    maxnum = 0
    ncalls = 0
    for b in range(NB):
        t0, t1 = b * TB, min((b + 1) * TB, NT)
        # window-major, then tile, then chunk
        groups = {}
        for t in range(t0, t1):
            for j in range(int(NCH[t])):
                groups.setdefault(win_of[(t, j)], []).append((t, j))
        cc = 0
        calls = []
        for w in sorted(groups):
            chunks = groups[w]
            for ci in range(0, len(chunks), MAXCALL):
                part = chunks[ci : ci + MAXCALL]
                num = len(part) * P
                for (t, j) in part:
                    col_of[(t, j)] = cc
                    cc += 1
                calls.append((w, idx_off, num, cc - len(part), ncalls))
                idx_off += 128 * (num // 16)
                maxnum = max(maxnum, num)
                ncalls += 1
        chb = cc
        chb_max = max(chb_max, chb)
        batch_meta.append((t0, t1, calls, slot_off, chb))
        slot_off += 128 * chb

    # per-device idx / slot streams following the (batch, call, chunk) layout
    idx_streams = []
    slot_streams = []
    for d in range(NCORES):
        iparts = []
        sparts = []
        for (t0, t1, calls, soff, chb) in batch_meta:
            sl = np.full((chb, P), 255.0, dtype=np.float32)
            for (w, ioff, num, goff, cid) in calls:
                nch_call = num // P
                loc = np.zeros((nch_call, P), dtype=np.int16)
                for k in range(nch_call):
                    # find which (t, j) this column is
                    pass
                iparts.append(loc)  # placeholder, filled below
            sparts.append(sl)
        idx_streams.append(iparts)
        slot_streams.append(sparts)

    # fill streams: iterate chunks once
    # column -> (t, j) map per batch
    for bi, (t0, t1, calls, soff, chb) in enumerate(batch_meta):
        colmap = {}
        for t in range(t0, t1):
            for j in range(int(NCH[t])):
                colmap[col_of[(t, j)]] = (t, j)
        for d in range(NCORES):
            sl = slot_streams[d][bi]
            ci_base = 0
            for k, (w, ioff, num, goff, cid) in enumerate(calls):
                nch_call = num // P
                loc = idx_streams[d][_call_flat_index(batch_meta, bi, k)]
                for c in range(nch_call):
                    col = goff + c
                    t, j = colmap[col]
                    rs = srows_of(d, t, j)
                    base = bases[w]
                    if rs is not None:
                        nn_ = len(rs)
                        loc[c, :nn_] = (rs - base).astype(np.int16)
                        a = gstart[d * NT + t]
                        sl[col, :nn_] = s_slot[a + j * P : a + j * P + nn_]
                    # dummies: loc 0 (= window base row), slot 255

    # pack idx streams into gather wire format: per call [128, num/16]
    idx_wire = []
    slot_wire = []
    for d in range(NCORES):
        iw = []
        for loc in idx_streams[d]:
            flat = loc.reshape(-1)  # [num]
            num = flat.shape[0]
            w16 = flat.reshape(num // 16, 16).T  # [16, num/16]
            w16 = np.tile(w16, (8, 1))  # [128, num/16]
            iw.append(w16.reshape(-1))
        idx_wire.append(np.concatenate(iw) if iw else np.zeros(16, np.int16))
        sw = []
        for sl in slot_streams[d]:
            sw.append(sl.T.reshape(-1))  # [P, chb] C-order
        slot_wire.append(np.concatenate(sw) if sw else np.zeros(128, np.float32))

    # per-tile ACT scale columns, [128, NT], f32
    dall_flat = np.zeros(V, np.float64)
    dall_flat[node_row] = dinv
    dall = dall_flat.reshape(NCORES, SHARD_PAD)
    dinv_col = dall.reshape(NCORES, NT, P).transpose(0, 2, 1).astype(np.float32)
    dinv2_col = (dall ** 2).reshape(NCORES, NT, P).transpose(0, 2, 1).astype(np.float32)
    sq = np.where(dall > 0, 1.0 / np.maximum(dall, 1e-30), 0.0)
    sqrtdeg_row = sq.astype(np.float32)  # [NCORES, SHARD_PAD]

    has_bias = bool(np.any(np.asarray(b1) != 0) or np.any(np.asarray(b2) != 0))

    meta = dict(
        N=N, D=D, SHARD=SHARD, SHARD_PAD=SHARD_PAD, NT=NT, V=V,
        bases=bases, wrows=wrows, NWIN=NWIN, NB=NB, NCH=NCH,
        batch_meta=batch_meta, col_of=col_of, has_bias=has_bias,
        chb_max=chb_max, maxnum=maxnum, node_row=node_row,
        idx_len=int(idx_wire[0].shape[0]), slot_len=int(slot_wire[0].shape[0]),
    )

    iota = np.tile(np.arange(P, dtype=ml_dtypes.bfloat16)[None, :], (P, 1))
    ident = np.eye(P, dtype=ml_dtypes.bfloat16)
    in_maps = []
    for d in range(NCORES):
        in_maps.append({
            "table1": table1,
            "selfrows": np.ascontiguousarray(table1[d * SHARD_PAD : (d + 1) * SHARD_PAD]),
            "idxs": np.ascontiguousarray(idx_wire[d]),
            "slots": np.ascontiguousarray(slot_wire[d]),
            "w1": np.asarray(W1, dtype=np.float32).astype(ml_dtypes.bfloat16),
            "w2": np.asarray(W2, dtype=np.float32).astype(ml_dtypes.bfloat16),
            "iota": iota,
            "ident": ident,
            "dinvc": np.ascontiguousarray(dinv_col[d]),
            "dinv2c": np.ascontiguousarray(dinv2_col[d]),
            "b1v": np.asarray(b1, dtype=np.float32)[None, :].astype(ml_dtypes.bfloat16),
            "b2v": np.asarray(b2, dtype=np.float32)[None, :].astype(ml_dtypes.bfloat16),
            "sqdeg": sqrtdeg_row[d][None, :],
        })
    return meta, in_maps


def _call_flat_index(batch_meta, bi, k):
    n = 0
    for i, (_, _, calls, _, _) in enumerate(batch_meta):
        if i == bi:
            return n + k
        n += len(calls)
    raise IndexError


def _build(meta):
    """Build the SPMD bass program."""
    D = meta["D"]
    NT, V = meta["NT"], meta["V"]
    SHARD_PAD = meta["SHARD_PAD"]
    NCH, batch_meta, col_of = meta["NCH"], meta["batch_meta"], meta["col_of"]
    bases, wrows = meta["bases"], meta["wrows"]
    has_bias = meta["has_bias"]
    chb_max, maxnum = meta["chb_max"], meta["maxnum"]
    bf16, f32, i16 = mybir.dt.bfloat16, mybir.dt.float32, mybir.dt.int16

    NQ = int(os.environ.get("GCN_NQ", "1"))
    stage = os.environ.get("GCN_STAGE", "full")  # l1 | nocc | full
    nc = bacc.Bacc("TRN2", target_bir_lowering=False, debug=False,
                   num_devices=NCORES)

    t_table1 = nc.dram_tensor("table1", [V, D], bf16, kind="ExternalInput")
    t_self = nc.dram_tensor("selfrows", [SHARD_PAD, D], bf16, kind="ExternalInput")
    t_idxs = nc.dram_tensor("idxs", [meta["idx_len"]], i16, kind="ExternalInput")
    t_slots = nc.dram_tensor("slots", [meta["slot_len"]], f32, kind="ExternalInput")
    t_w1 = nc.dram_tensor("w1", [D, D], bf16, kind="ExternalInput")
    t_w2 = nc.dram_tensor("w2", [D, D], bf16, kind="ExternalInput")
    t_iota = nc.dram_tensor("iota", [P, P], bf16, kind="ExternalInput")
    t_ident = nc.dram_tensor("ident", [P, P], bf16, kind="ExternalInput")
    t_dinvc = nc.dram_tensor("dinvc", [P, NT], f32, kind="ExternalInput")
    t_dinv2c = nc.dram_tensor("dinv2c", [P, NT], f32, kind="ExternalInput")
    t_b1 = nc.dram_tensor("b1v", [1, D], bf16, kind="ExternalInput")
    t_b2 = nc.dram_tensor("b2v", [1, D], bf16, kind="ExternalInput")
    t_sqdeg = nc.dram_tensor("sqdeg", [1, SHARD_PAD], f32, kind="ExternalInput")

    t2shard = nc.dram_tensor("t2shard", [SHARD_PAD, D], bf16, kind="Internal")
    t2full = nc.dram_tensor("t2full", [V, D], bf16, kind="Internal",
                            addr_space="Shared")
    t_out = nc.dram_tensor("out", [SHARD_PAD, D], f32, kind="ExternalOutput")

    with tile.TileContext(nc) as tc:
        with (
            tc.tile_pool(name="const", bufs=1) as cp,
            tc.tile_pool(name="gp", bufs=3) as gp,
            tc.tile_pool(name="ip", bufs=8) as ip,
            tc.tile_pool(name="slp", bufs=2) as slp,
            tc.tile_pool(name="stp", bufs=4) as stp,
            tc.tile_pool(name="ohp", bufs=6) as ohp,
            tc.tile_pool(name="ptp", bufs=8) as ptp,
            tc.tile_pool(name="op", bufs=4) as op,
            tc.tile_pool(name="pspt", bufs=4, space="PSUM") as pspt,
            tc.tile_pool(name="psh", bufs=2, space="PSUM") as psh,
        ):
            w1_sb = cp.tile([D, D], bf16, tag="w1")
            w2_sb = cp.tile([D, D], bf16, tag="w2")
            iota_sb = cp.tile([P, P], bf16, tag="iota")
            ident_sb = cp.tile([P, P], bf16, tag="ident")
            dinvc_sb = cp.tile([P, NT], f32, tag="dinvc")
            dinv2c_sb = cp.tile([P, NT], f32, tag="dinv2c")
            nc.sync.dma_start(out=w1_sb[:], in_=t_w1[:])
            nc.sync.dma_start(out=w2_sb[:], in_=t_w2[:])
            nc.sync.dma_start(out=iota_sb[:], in_=t_iota[:])
            nc.sync.dma_start(out=ident_sb[:], in_=t_ident[:])
            nc.sync.dma_start(out=dinvc_sb[:], in_=t_dinvc[:])
            nc.sync.dma_start(out=dinv2c_sb[:], in_=t_dinv2c[:])
            if has_bias:
                b1_sb = cp.tile([1, D], bf16, tag="b1")
                b2_sb = cp.tile([1, D], bf16, tag="b2")
                sq_sb = cp.tile([1, SHARD_PAD], f32, tag="sq")
                nc.sync.dma_start(out=b1_sb[:], in_=t_b1[:])
                nc.sync.dma_start(out=b2_sb[:], in_=t_b2[:])
                nc.sync.dma_start(out=sq_sb[:], in_=t_sqdeg[:])

            layers = (0,) if stage == "l1" else (0, 1)
            for layer in layers:
                table = t_table1 if (layer == 0 or stage == "nocc") else t2full
                selftab = t_self if layer == 0 else t2shard
                w_sb = w1_sb if layer == 0 else w2_sb
                for (t0, t1, calls, slot_off, chb) in batch_meta:
                    g = gp.tile([P, chb_max, D], bf16, tag="g")
                    for (w, idx_off, num, goff, cid) in calls:
                        ix = ip.tile([P, maxnum // 16], i16, tag="idx")
                        ncols16 = num // 16
                        nc.sync.dma_start(
                            out=ix[:, :ncols16],
                            in_=t_idxs[idx_off : idx_off + 128 * ncols16]
                            .rearrange("(p c) -> p c", p=128),
                        )
                        base = int(bases[w])
                        nc.gpsimd.dma_gather(
                            g[:, goff : goff + num // P, :],
                            table[base : base + wrows, :],
                            ix[:, :ncols16],
                            num, num, D, single_packet=False,
                            queue_num=cid % NQ,
                        )
                    sl = slp.tile([P, chb_max], f32, tag="slot")
                    nc.sync.dma_start(
                        out=sl[:, :chb],
                        in_=t_slots[slot_off : slot_off + 128 * chb]
                        .rearrange("(p c) -> p c", p=128),
                    )
                    for t in range(t0, t1):
                        si = t - t0
                        q = si % 4
                        if q == 0:
                            pt = pspt.tile([P, 4 * P], f32, tag="pt")
                            hb = psh.tile([P, 4 * P], f32, tag="hb")
                        # self-loop contribution: identity-rhs matmul
                        st = stp.tile([P, D], bf16, tag="st")
                        nc.sync.dma_start(
                            out=st[:], in_=selftab[t * P : (t + 1) * P, :])
                        nchunks_t = int(NCH[t])
                        nc.tensor.matmul(
                            out=pt[:, q * P : (q + 1) * P],
                            lhsT=st[:], rhs=ident_sb[:],
                            start=True, stop=(nchunks_t == 0),
                        )
                        for j in range(nchunks_t):
                            gcol = col_of[(t, j)]
                            s = ohp.tile([P, P], bf16, tag="oh")
                            nc.vector.tensor_scalar(
                                s[:], iota_sb[:],
                                sl[:, gcol : gcol + 1], None,
                                mybir.AluOpType.is_equal,
                            )
                            nc.tensor.matmul(
                                out=pt[:, q * P : (q + 1) * P],
                                lhsT=g[:, gcol, :],
                                rhs=s[:],
                                start=False,
                                stop=(j == nchunks_t - 1),
                            )
                        ptsb = ptp.tile([P, P], bf16, tag="ptsb")
                        nc.scalar.copy(out=ptsb[:], in_=pt[:, q * P : (q + 1) * P])
                        if has_bias:
                            bv = b1_sb if layer == 0 else b2_sb
                            nc.tensor.matmul(
                                out=hb[:, q * P : (q + 1) * P],
                                lhsT=sq_sb[:, t * P : (t + 1) * P],
                                rhs=bv[:],
                                start=True, stop=False,
                            )
                        nc.tensor.matmul(
                            out=hb[:, q * P : (q + 1) * P],
                            lhsT=ptsb[:], rhs=w_sb[:],
                            start=not has_bias, stop=True,
                        )
                        if layer == 0 and stage == "l1":
                            o = op.tile([P, P], f32, tag="o2")
                            nc.scalar.activation(
                                out=o[:], in_=hb[:, q * P : (q + 1) * P],
                                func=mybir.ActivationFunctionType.Relu,
                                scale=dinv2c_sb[:, t : t + 1],
                            )
                            nc.sync.dma_start(
                                out=t_out[t * P : (t + 1) * P, :], in_=o[:])
                        elif layer == 0:
                            o = op.tile([P, P], bf16, tag="o1")
                            nc.scalar.activation(
                                out=o[:], in_=hb[:, q * P : (q + 1) * P],
                                func=mybir.ActivationFunctionType.Relu,
                                scale=dinv2c_sb[:, t : t + 1],
                            )
                            nc.sync.dma_start(
                                out=t2shard[t * P : (t + 1) * P, :], in_=o[:])
                        else:
                            o = op.tile([P, P], f32, tag="o2")
                            nc.scalar.activation(
                                out=o[:], in_=hb[:, q * P : (q + 1) * P],
                                func=mybir.ActivationFunctionType.Relu,
                                scale=dinvc_sb[:, t : t + 1],
                            )
                            nc.sync.dma_start(
                                out=t_out[t * P : (t + 1) * P, :], in_=o[:])
                if layer == 0 and stage == "full" and len(layers) > 1:
                    nc.gpsimd.collective_compute(
                        "AllGather", mybir.AluOpType.bypass,
                        replica_groups=[list(range(NCORES))],
                        ins=[t2shard[:]], outs=[t2full[:]],
                    )
    nc.compile()
    return nc


def kernel(x, W1, b1, W2, b2, edge_index):
    global LAST_RESULTS
    x = np.asarray(x)
    N = x.shape[0]
    meta, in_maps = _prep(x, W1, b1, W2, b2, edge_index)
    nc = _build(meta)
    node_row = meta["node_row"]
    if os.environ.get("GCN_SIM", "0") == "1":
        from concourse.bass_interp import MultiCoreSim

        sim = MultiCoreSim(nc, num_cores=NCORES, trace=False,
                           require_finite=False, require_nnan=False)
        cores = [sim.cores[i] for i in sorted(sim.cores)]
        for d, core in enumerate(cores):
            for k, v in in_maps[d].items():
                core.tensor(k)[:] = v
        sim.simulate(check_with_hw=False)
        shards = [np.array(core.tensor("out")) for core in cores]
        return np.concatenate(shards, axis=0)[node_row].astype(np.float32)
    trace = bool(int(os.environ.get("GCN_TRACE", "0")))
    ncr = int(os.environ.get("GCN_CORES", str(NCORES)))
    res = run_bass_kernel_spmd(nc, in_maps[:ncr], core_ids=list(range(ncr)),
                               trace=trace)
    LAST_RESULTS = res
    zero = np.zeros((meta["SHARD_PAD"], meta["D"]), np.float32)
    shards = [res.results[d]["out"] if d < len(res.results) else zero
              for d in range(NCORES)]
    return np.concatenate(shards, axis=0)[node_row].astype(np.float32)



# revision 7
# speedup vs baseline: 2.7375x; 2.7375x over previous
"""GCN (2-layer GCNConv + ReLU) Trainium2 Bass kernel, 8-core SPMD.

Math: with A = D^-1/2 (Adj + I) D^-1/2 (self loops added, symmetric norm),
  h1 = relu(A @ (x @ W1) + b1)
  h2 = relu(A @ (h1 @ W2) + b2)
Factorization used on device (norm_e = dinv[src]*dinv[dst], segment_sum
commutes with the weight GEMM):
  table1 = dinv * x                     (host, bf16)
  PT[t]  = selfrows[t] (identity mm) + sum_e table[src_e] x onehot(slot_e)
  psum   = PT^T @ W + sqrtdeg*b                     (PE)
  layer1 out (= table2 = dinv*h1) = relu(dinv^2 * psum)   (ACT, bf16)
  layer2 out (= h2)               = relu(dinv   * psum)   (ACT, f32)
Between layers an AllGather shares table2 across the 8 cores.

Sharding: nodes split in contiguous ranges; node g lives on device g//SHARD,
local row g%SHARD; per-device rows padded to SHARD_PAD (mult of 128).
Non-self edges partitioned by dst device, sorted by (dst tile, src row) and
cut into 128-edge chunks; each chunk is gathered from one 32768-row window
of the table (overlapping windows, chosen per chunk from the data so the
int16 gather indices fit). Self loops are handled without any gather: an
identity-rhs matmul adds the local shard's 128 rows per dst tile.
"""

import os
import sys

sys.path.insert(0, "/opt/trn_rl_repo")

import numpy as np
import ml_dtypes

from concourse import bass, bacc, mybir, tile
from concourse.bass_utils import run_bass_kernel_spmd

NCORES = 8
P = 128
WROWS = 32768          # int16 gather index limit (rows per window)
TB = 8                 # tiles per gather batch
MAXCALL = 48           # max chunks per dma_gather call (SWDGE ring safety)

LAST_RESULTS = None    # test harness peeks at this for profile info


def _ceil_div(a, b):
    return -(-a // b)


def _prep(x, W1, b1, W2, b2, edge_index):
    """Host-side graph preprocessing -> (meta, per-device input maps)."""
    N, D = x.shape
    SHARD = _ceil_div(N, NCORES)
    SHARD_PAD = _ceil_div(SHARD, P) * P
    NT = SHARD_PAD // P
    V = NCORES * SHARD_PAD

    # gather windows (overlapping, data-assigned per chunk)
    if V <= WROWS:
        bases = np.array([0], dtype=np.int64)
        wrows = V
    else:
        nw = max(2, _ceil_div(V - WROWS, 16896) + 1)
        stride = _ceil_div(V - WROWS, nw - 1)
        bases = np.array(
            sorted({min(w * stride, V - WROWS) for w in range(nw)}), dtype=np.int64
        )
        wrows = WROWS
    NWIN = len(bases)

    src = np.asarray(edge_index[0], dtype=np.int64)
    dst = np.asarray(edge_index[1], dtype=np.int64)

    deg = (np.bincount(dst, minlength=N) + 1).astype(np.float64)  # + self loop
    dinv = 1.0 / np.sqrt(deg)

    # in-degree-balanced node -> table-row permutation (snake deal over all
    # NCORES*NT 128-node bins) so every (device, tile) has ~equal edge count
    # and chunk counts hit the ceil with minimal padding.
    NBINS = NCORES * NT
    indeg = np.bincount(dst, minlength=N)
    by_deg = np.argsort(-indeg, kind="stable")
    node_row = np.empty(N, dtype=np.int64)
    bin_fill = np.zeros(NBINS, dtype=np.int64)
    fwd = np.arange(NBINS)
    for r in range(_ceil_div(N, NBINS)):
        sl_nodes = by_deg[r * NBINS : (r + 1) * NBINS]
        bins = fwd[: len(sl_nodes)] if r % 2 == 0 else (NBINS - 1 - fwd)[: len(sl_nodes)]
        d_of = bins % NCORES
        t_of = bins // NCORES
        node_row[sl_nodes] = d_of * SHARD_PAD + t_of * P + bin_fill[bins]
        bin_fill[bins] += 1
    assert bin_fill.max() <= P

    # layer-1 gather table: dinv * x at permuted rows, padded to V rows, bf16
    table1 = np.zeros((V, D), dtype=ml_dtypes.bfloat16)
    scaled = (x.astype(np.float64) * dinv[:, None]).astype(np.float32)
    table1[node_row] = scaled.astype(ml_dtypes.bfloat16)

    # edges -> (device, tile, slot, srow), sorted by (dev, tile, srow)
    drow = node_row[dst]
    dev = drow // SHARD_PAD
    ltile = (drow % SHARD_PAD) // P
    slot = (drow % P).astype(np.int64)
    srow = node_row[src]
    order = np.lexsort((srow, ltile, dev))
    s_dev, s_tile = dev[order], ltile[order]
    s_srow, s_slot = srow[order], slot[order]

    key = s_dev * NT + s_tile
    cnt = np.bincount(key, minlength=NCORES * NT).reshape(NCORES, NT)
    NCH = _ceil_div(cnt, P).max(axis=0)  # [NT] chunks per tile (can be 0)
    gstart = np.zeros(NCORES * NT + 1, dtype=np.int64)
    np.cumsum(cnt.reshape(-1), out=gstart[1:])

    # per-(tile, chunk) window assignment from LO/HI over devices
    def srows_of(d, t, j):
        a = gstart[d * NT + t]
        c = cnt[d, t]
        lo_i, hi_i = j * P, min((j + 1) * P, c)
        if lo_i >= hi_i:
            return None
        return s_srow[a + lo_i : a + hi_i]

    win_of = {}
    for t in range(NT):
        for j in range(int(NCH[t])):
            lo, hi = V, -1
            for d in range(NCORES):
                rs = srows_of(d, t, j)
                if rs is not None:
                    lo = min(lo, int(rs[0]))
                    hi = max(hi, int(rs[-1]))
            ok = np.where((bases <= lo) & (hi < bases + wrows))[0]
            assert len(ok) > 0, f"no window fits tile {t} chunk {j}: [{lo},{hi}]"
            mid = (lo + hi) / 2
            win_of[(t, j)] = int(ok[np.argmin(np.abs(bases[ok] + wrows / 2 - mid))])

    # batch / call / column layout
    NB = _ceil_div(NT, TB)
    batch_meta = []   # per batch: (t0, t1, calls, slot_off, chb)
    col_of = {}       # (t, j) -> column within its batch's g tile
    idx_parts, slot_parts = [], []
    idx_off = 0
    slot_off = 0
    chb_max = 0
    maxnum = 0
    ncalls = 0
    for b in range(NB):
        t0, t1 = b * TB, min((b + 1) * TB, NT)
        # window-major, then tile, then chunk
        groups = {}
        for t in range(t0, t1):
            for j in range(int(NCH[t])):
                groups.setdefault(win_of[(t, j)], []).append((t, j))
        cc = 0
        calls = []
        for w in sorted(groups):
            chunks = groups[w]
            for ci in range(0, len(chunks), MAXCALL):
                part = chunks[ci : ci + MAXCALL]
                num = len(part) * P
                for (t, j) in part:
                    col_of[(t, j)] = cc
                    cc += 1
                calls.append((w, idx_off, num, cc - len(part), ncalls))
                idx_off += 128 * (num // 16)
                maxnum = max(maxnum, num)
                ncalls += 1
        chb = cc
        chb_max = max(chb_max, chb)
        batch_meta.append((t0, t1, calls, slot_off, chb))
        slot_off += 128 * chb

    # per-device idx / slot streams following the (batch, call, chunk) layout
    idx_streams = []
    slot_streams = []
    for d in range(NCORES):
        iparts = []
        sparts = []
        for (t0, t1, calls, soff, chb) in batch_meta:
            sl = np.full((chb, P), 255.0, dtype=np.float32)
            for (w, ioff, num, goff, cid) in calls:
                nch_call = num // P
                loc = np.zeros((nch_call, P), dtype=np.int16)
                for k in range(nch_call):
                    # find which (t, j) this column is
                    pass
                iparts.append(loc)  # placeholder, filled below
            sparts.append(sl)
        idx_streams.append(iparts)
        slot_streams.append(sparts)

    # fill streams: iterate chunks once
    # column -> (t, j) map per batch
    for bi, (t0, t1, calls, soff, chb) in enumerate(batch_meta):
        colmap = {}
        for t in range(t0, t1):
            for j in range(int(NCH[t])):
                colmap[col_of[(t, j)]] = (t, j)
        for d in range(NCORES):
            sl = slot_streams[d][bi]
            ci_base = 0
            for k, (w, ioff, num, goff, cid) in enumerate(calls):
                nch_call = num // P
                loc = idx_streams[d][_call_flat_index(batch_meta, bi, k)]
                for c in range(nch_call):
                    col = goff + c
                    t, j = colmap[col]
                    rs = srows_of(d, t, j)
                    base = bases[w]
                    if rs is not None:
                        nn_ = len(rs)
                        loc[c, :nn_] = (rs - base).astype(np.int16)
                        a = gstart[d * NT + t]
                        sl[col, :nn_] = s_slot[a + j * P : a + j * P + nn_]
                    # dummies: loc 0 (= window base row), slot 255

    # pack idx streams into gather wire format: per call [128, num/16]
    idx_wire = []
    slot_wire = []
    for d in range(NCORES):
        iw = []
        for loc in idx_streams[d]:
            flat = loc.reshape(-1)  # [num]
            num = flat.shape[0]
            w16 = flat.reshape(num // 16, 16).T  # [16, num/16]
            w16 = np.tile(w16, (8, 1))  # [128, num/16]
            iw.append(w16.reshape(-1))
        idx_wire.append(np.concatenate(iw) if iw else np.zeros(16, np.int16))
        sw = []
        for sl in slot_streams[d]:
            sw.append(sl.T.reshape(-1))  # [P, chb] C-order
        slot_wire.append(np.concatenate(sw) if sw else np.zeros(128, np.float32))

    # per-tile ACT scale columns, [128, NT], f32
    dall_flat = np.zeros(V, np.float64)
    dall_flat[node_row] = dinv
    dall = dall_flat.reshape(NCORES, SHARD_PAD)
    dinv_col = dall.reshape(NCORES, NT, P).transpose(0, 2, 1).astype(np.float32)
    dinv2_col = (dall ** 2).reshape(NCORES, NT, P).transpose(0, 2, 1).astype(np.float32)
    sq = np.where(dall > 0, 1.0 / np.maximum(dall, 1e-30), 0.0)
    sqrtdeg_row = sq.astype(np.float32)  # [NCORES, SHARD_PAD]

    has_bias = bool(np.any(np.asarray(b1) != 0) or np.any(np.asarray(b2) != 0))

    meta = dict(
        N=N, D=D, SHARD=SHARD, SHARD_PAD=SHARD_PAD, NT=NT, V=V,
        bases=bases, wrows=wrows, NWIN=NWIN, NB=NB, NCH=NCH,
        batch_meta=batch_meta, col_of=col_of, has_bias=has_bias,
        chb_max=chb_max, maxnum=maxnum, node_row=node_row,
        idx_len=int(idx_wire[0].shape[0]), slot_len=int(slot_wire[0].shape[0]),
    )

    iota = np.tile(np.arange(P, dtype=ml_dtypes.bfloat16)[None, :], (P, 1))
    ident = np.eye(P, dtype=ml_dtypes.bfloat16)
    in_maps = []
    for d in range(NCORES):
        in_maps.append({
            "table1": table1,
            "selfrows": np.ascontiguousarray(table1[d * SHARD_PAD : (d + 1) * SHARD_PAD]),
            "idxs": np.ascontiguousarray(idx_wire[d]),
            "slots": np.ascontiguousarray(slot_wire[d]),
            "w1": np.asarray(W1, dtype=np.float32).astype(ml_dtypes.bfloat16),
            "w2": np.asarray(W2, dtype=np.float32).astype(ml_dtypes.bfloat16),
            "iota": iota,
            "ident": ident,
            "dinvc": np.ascontiguousarray(dinv_col[d]),
            "dinv2c": np.ascontiguousarray(dinv2_col[d]),
            "b1v": np.asarray(b1, dtype=np.float32)[None, :].astype(ml_dtypes.bfloat16),
            "b2v": np.asarray(b2, dtype=np.float32)[None, :].astype(ml_dtypes.bfloat16),
            "sqdeg": sqrtdeg_row[d][None, :],
        })
    return meta, in_maps


def _call_flat_index(batch_meta, bi, k):
    n = 0
    for i, (_, _, calls, _, _) in enumerate(batch_meta):
        if i == bi:
            return n + k
        n += len(calls)
    raise IndexError


def _build(meta):
    """Build the SPMD bass program."""
    D = meta["D"]
    NT, V = meta["NT"], meta["V"]
    SHARD_PAD = meta["SHARD_PAD"]
    NCH, batch_meta, col_of = meta["NCH"], meta["batch_meta"], meta["col_of"]
    bases, wrows = meta["bases"], meta["wrows"]
    has_bias = meta["has_bias"]
    chb_max, maxnum = meta["chb_max"], meta["maxnum"]
    bf16, f32, i16 = mybir.dt.bfloat16, mybir.dt.float32, mybir.dt.int16

    NQ = int(os.environ.get("GCN_NQ", "4"))
    stage = os.environ.get("GCN_STAGE", "full")  # l1 | nocc | full
    nc = bacc.Bacc("TRN2", target_bir_lowering=False, debug=False,
                   num_devices=NCORES, num_swdge_queues=NQ)

    t_table1 = nc.dram_tensor("table1", [V, D], bf16, kind="ExternalInput")
    t_self = nc.dram_tensor("selfrows", [SHARD_PAD, D], bf16, kind="ExternalInput")
    t_idxs = nc.dram_tensor("idxs", [meta["idx_len"]], i16, kind="ExternalInput")
    t_slots = nc.dram_tensor("slots", [meta["slot_len"]], f32, kind="ExternalInput")
    t_w1 = nc.dram_tensor("w1", [D, D], bf16, kind="ExternalInput")
    t_w2 = nc.dram_tensor("w2", [D, D], bf16, kind="ExternalInput")
    t_iota = nc.dram_tensor("iota", [P, P], bf16, kind="ExternalInput")
    t_ident = nc.dram_tensor("ident", [P, P], bf16, kind="ExternalInput")
    t_dinvc = nc.dram_tensor("dinvc", [P, NT], f32, kind="ExternalInput")
    t_dinv2c = nc.dram_tensor("dinv2c", [P, NT], f32, kind="ExternalInput")
    t_b1 = nc.dram_tensor("b1v", [1, D], bf16, kind="ExternalInput")
    t_b2 = nc.dram_tensor("b2v", [1, D], bf16, kind="ExternalInput")
    t_sqdeg = nc.dram_tensor("sqdeg", [1, SHARD_PAD], f32, kind="ExternalInput")

    t2shard = nc.dram_tensor("t2shard", [SHARD_PAD, D], bf16, kind="Internal")
    t2full = nc.dram_tensor("t2full", [V, D], bf16, kind="Internal",
                            addr_space="Shared")
    t_out = nc.dram_tensor("out", [SHARD_PAD, D], f32, kind="ExternalOutput")

    with tile.TileContext(nc) as tc:
        with (
            tc.tile_pool(name="const", bufs=1) as cp,
            tc.tile_pool(name="gp", bufs=3) as gp,
            tc.tile_pool(name="ip", bufs=8) as ip,
            tc.tile_pool(name="slp", bufs=2) as slp,
            tc.tile_pool(name="stp", bufs=4) as stp,
            tc.tile_pool(name="ohp", bufs=6) as ohp,
            tc.tile_pool(name="ptp", bufs=8) as ptp,
            tc.tile_pool(name="op", bufs=4) as op,
            tc.tile_pool(name="pspt", bufs=4, space="PSUM") as pspt,
            tc.tile_pool(name="psh", bufs=2, space="PSUM") as psh,
        ):
            w1_sb = cp.tile([D, D], bf16, tag="w1")
            w2_sb = cp.tile([D, D], bf16, tag="w2")
            iota_sb = cp.tile([P, P], bf16, tag="iota")
            ident_sb = cp.tile([P, P], bf16, tag="ident")
            dinvc_sb = cp.tile([P, NT], f32, tag="dinvc")
            dinv2c_sb = cp.tile([P, NT], f32, tag="dinv2c")
            nc.sync.dma_start(out=w1_sb[:], in_=t_w1[:])
            nc.sync.dma_start(out=w2_sb[:], in_=t_w2[:])
            nc.sync.dma_start(out=iota_sb[:], in_=t_iota[:])
            nc.sync.dma_start(out=ident_sb[:], in_=t_ident[:])
            nc.sync.dma_start(out=dinvc_sb[:], in_=t_dinvc[:])
            nc.sync.dma_start(out=dinv2c_sb[:], in_=t_dinv2c[:])
            if has_bias:
                b1_sb = cp.tile([1, D], bf16, tag="b1")
                b2_sb = cp.tile([1, D], bf16, tag="b2")
                sq_sb = cp.tile([1, SHARD_PAD], f32, tag="sq")
                nc.sync.dma_start(out=b1_sb[:], in_=t_b1[:])
                nc.sync.dma_start(out=b2_sb[:], in_=t_b2[:])
                nc.sync.dma_start(out=sq_sb[:], in_=t_sqdeg[:])

            layers = (0,) if stage == "l1" else (0, 1)
            for layer in layers:
                table = t_table1 if (layer == 0 or stage == "nocc") else t2full
                selftab = t_self if layer == 0 else t2shard
                w_sb = w1_sb if layer == 0 else w2_sb
                for (t0, t1, calls, slot_off, chb) in batch_meta:
                    g = gp.tile([P, chb_max, D], bf16, tag="g")
                    for (w, idx_off, num, goff, cid) in calls:
                        ix = ip.tile([P, maxnum // 16], i16, tag="idx")
                        ncols16 = num // 16
                        nc.sync.dma_start(
                            out=ix[:, :ncols16],
                            in_=t_idxs[idx_off : idx_off + 128 * ncols16]
                            .rearrange("(p c) -> p c", p=128),
                        )
                        base = int(bases[w])
                        nc.gpsimd.dma_gather(
                            g[:, goff : goff + num // P, :],
                            table[base : base + wrows, :],
                            ix[:, :ncols16],
                            num, num, D, single_packet=False,
                            queue_num=cid % NQ,
                        )
                    sl = slp.tile([P, chb_max], f32, tag="slot")
                    nc.sync.dma_start(
                        out=sl[:, :chb],
                        in_=t_slots[slot_off : slot_off + 128 * chb]
                        .rearrange("(p c) -> p c", p=128),
                    )
                    for t in range(t0, t1):
                        si = t - t0
                        q = si % 4
                        if q == 0:
                            pt = pspt.tile([P, 4 * P], f32, tag="pt")
                            hb = psh.tile([P, 4 * P], f32, tag="hb")
                        # self-loop contribution: identity-rhs matmul
                        st = stp.tile([P, D], bf16, tag="st")
                        nc.sync.dma_start(
                            out=st[:], in_=selftab[t * P : (t + 1) * P, :])
                        nchunks_t = int(NCH[t])
                        nc.tensor.matmul(
                            out=pt[:, q * P : (q + 1) * P],
                            lhsT=st[:], rhs=ident_sb[:],
                            start=True, stop=(nchunks_t == 0),
                        )
                        for j in range(nchunks_t):
                            gcol = col_of[(t, j)]
                            s = ohp.tile([P, P], bf16, tag="oh")
                            nc.vector.tensor_scalar(
                                s[:], iota_sb[:],
                                sl[:, gcol : gcol + 1], None,
                                mybir.AluOpType.is_equal,
                            )
                            nc.tensor.matmul(
                                out=pt[:, q * P : (q + 1) * P],
                                lhsT=g[:, gcol, :],
                                rhs=s[:],
                                start=False,
                                stop=(j == nchunks_t - 1),
                            )
                        ptsb = ptp.tile([P, P], bf16, tag="ptsb")
                        nc.scalar.copy(out=ptsb[:], in_=pt[:, q * P : (q + 1) * P])
                        if has_bias:
                            bv = b1_sb if layer == 0 else b2_sb
                            nc.tensor.matmul(
                                out=hb[:, q * P : (q + 1) * P],
                                lhsT=sq_sb[:, t * P : (t + 1) * P],
                                rhs=bv[:],
                                start=True, stop=False,
                            )
                        nc.tensor.matmul(
                            out=hb[:, q * P : (q + 1) * P],
                            lhsT=ptsb[:], rhs=w_sb[:],
                            start=not has_bias, stop=True,
                        )
                        if layer == 0 and stage == "l1":
                            o = op.tile([P, P], f32, tag="o2")
                            nc.scalar.activation(
                                out=o[:], in_=hb[:, q * P : (q + 1) * P],
                                func=mybir.ActivationFunctionType.Relu,
                                scale=dinv2c_sb[:, t : t + 1],
                            )
                            nc.sync.dma_start(
                                out=t_out[t * P : (t + 1) * P, :], in_=o[:])
                        elif layer == 0:
                            o = op.tile([P, P], bf16, tag="o1")
                            nc.scalar.activation(
                                out=o[:], in_=hb[:, q * P : (q + 1) * P],
                                func=mybir.ActivationFunctionType.Relu,
                                scale=dinv2c_sb[:, t : t + 1],
                            )
                            nc.sync.dma_start(
                                out=t2shard[t * P : (t + 1) * P, :], in_=o[:])
                        else:
                            o = op.tile([P, P], f32, tag="o2")
                            nc.scalar.activation(
                                out=o[:], in_=hb[:, q * P : (q + 1) * P],
                                func=mybir.ActivationFunctionType.Relu,
                                scale=dinvc_sb[:, t : t + 1],
                            )
                            nc.sync.dma_start(
                                out=t_out[t * P : (t + 1) * P, :], in_=o[:])
                if layer == 0 and stage == "full" and len(layers) > 1:
                    nc.gpsimd.collective_compute(
                        "AllGather", mybir.AluOpType.bypass,
                        replica_groups=[list(range(NCORES))],
                        ins=[t2shard[:]], outs=[t2full[:]],
                    )
    nc.compile()
    return nc


def kernel(x, W1, b1, W2, b2, edge_index):
    global LAST_RESULTS
    x = np.asarray(x)
    N = x.shape[0]
    meta, in_maps = _prep(x, W1, b1, W2, b2, edge_index)
    nc = _build(meta)
    node_row = meta["node_row"]
    if os.environ.get("GCN_SIM", "0") == "1":
        from concourse.bass_interp import MultiCoreSim

        sim = MultiCoreSim(nc, num_cores=NCORES, trace=False,
                           require_finite=False, require_nnan=False)
        cores = [sim.cores[i] for i in sorted(sim.cores)]
        for d, core in enumerate(cores):
            for k, v in in_maps[d].items():
                core.tensor(k)[:] = v
        sim.simulate(check_with_hw=False)
        shards = [np.array(core.tensor("out")) for core in cores]
        return np.concatenate(shards, axis=0)[node_row].astype(np.float32)
    trace = bool(int(os.environ.get("GCN_TRACE", "0")))
    ncr = int(os.environ.get("GCN_CORES", str(NCORES)))
    res = run_bass_kernel_spmd(nc, in_maps[:ncr], core_ids=list(range(ncr)),
                               trace=trace)
    LAST_RESULTS = res
    zero = np.zeros((meta["SHARD_PAD"], meta["D"]), np.float32)
    shards = [res.results[d]["out"] if d < len(res.results) else zero
              for d in range(NCORES)]
    return np.concatenate(shards, axis=0)[node_row].astype(np.float32)

